# revision 17
# baseline (speedup 1.0000x reference)
"""Trainium2 Bass kernel for nn_BlockAttentionResidual (block attention + BitNet MLP).

Sharding: sequence-block data parallelism. Block attention is independent per
512-token block, so each of the 8 cores owns 1024 contiguous tokens (2 blocks)
of one batch element and runs the ENTIRE layer on them with zero collectives.
  core c -> batch c//4, tokens [(c%4)*1024, (c%4+1)*1024)

All weight quantization happens on the host (ternary weights shipped as bf16,
exact), ffn_up ships as fp16 (~= f32r precision at half the DMA bytes); compact
rope tables; o_proj fused with the ffn rmsnorm; K-side dequant folded into the
softmax exp scale.  Activation tensors that cross phase boundaries are split
per 128-token tile so each consumer phase starts as soon as its own tile is
ready; pure loads issue on the scalar HWDGE queue so they never queue behind
compute-dependent transposes/stores on sync.

Numerics: bitlinear matmuls run on integer-valued bf16 operands (act-quant ints
in [-128,127] x ternary weights are exact in bf16; PSUM accumulates fp32);
attention in bf16 with fp32 accum; softmax skips max-subtraction (|score*scale|
~ 3); ffn_up in fp16 (10 mantissa bits = f32r).
"""

import numpy as np
import ml_dtypes

import concourse.bass as bass
import concourse.mybir as mybir
import concourse.tile as tile
from concourse import bacc
from concourse.bass_utils import run_bass_kernel_spmd

F32 = mybir.dt.float32
BF16 = mybir.dt.bfloat16
F16 = mybir.dt.float16
AX = mybir.AxisListType
OP = mybir.AluOpType
ACTF = mybir.ActivationFunctionType

# model dims
H = 2048
NH = 16
HD = 128
INTER = 4096        # reference INTER = 2*H; ffn_up rows = 2*INTER = 8192
EPS = 1e-5
THETA = 10000.0
B, S = 2, 4096
NCORES = 8
R = 1024            # tokens per core (= 2 attention blocks of 512)
NT = R // 128       # 8 token tiles per core
MAGIC = np.float32(1.5 * 2 ** 23)   # fp32 round-to-nearest-even magic
SCALE_QK = float(HD ** -0.5)

# strip order for the interleaved qkv stream: per head-group g the q, k, v
# strips (cols g*512 | 2048+g*512 | 4096+g*512 of w_qkv^T)
QKV_ORDER = [0, 4, 8, 1, 5, 9, 2, 6, 10, 3, 7, 11]
UP_ORDER = [0, 8, 1, 9, 2, 10, 3, 11, 4, 12, 5, 13, 6, 14, 7, 15]


def build_program(skip_anw, skip_fnw):
    nc = bacc.Bacc(None, target_bir_lowering=False)

    # ---- I/O ----
    x_in = nc.declare_dram_parameter("x_sh", [R, H], F32, isOutput=False)
    cos_in = nc.declare_dram_parameter("cosb", [NT, 128, 64], F32, isOutput=False)
    sin_in = nc.declare_dram_parameter("sinb", [NT, 128, 64], F32, isOutput=False)
    anw_in = nc.declare_dram_parameter("attn_norm_w", [H], F32, isOutput=False)
    fnw_in = nc.declare_dram_parameter("ffn_norm_w", [H], F32, isOutput=False)
    dqs_in = nc.declare_dram_parameter("dqs", [4], F32, isOutput=False)
    wqkv_in = nc.declare_dram_parameter("wqkv3", [12, 16, 128, 512], BF16, isOutput=False)
    wo_in = nc.declare_dram_parameter("wo3", [4, 16, 128, 512], BF16, isOutput=False)
    wup_in = nc.declare_dram_parameter("wup_h", [16, 16, 128, 512], F16, isOutput=False)
    wdn_in = nc.declare_dram_parameter("wdn3", [4, 32, 128, 512], BF16, isOutput=False)
    out_d = nc.declare_dram_parameter("out_sh", [R, H], F32, isOutput=True)

    # ---- internal DRAM scratch ----
    x1_d = nc.dram_tensor("x1_d", [NT, 128, H], F32)
    act_d = nc.dram_tensor("act_d", [NT, 128, INTER], F16)

    with tile.TileContext(nc) as tc:
        perm = tc.alloc_tile_pool(name="perm", bufs=1)
        magic_t = perm.tile([128, 1], F32)
        nc.vector.memset(magic_t[:], float(MAGIC))
        magic_ap = magic_t[:]
        dq_b = perm.tile([128, 4], F32)
        dqa_all = perm.tile([128, NT], F32)    # per-token dequant for q/v
        e_scale = perm.tile([128, NT], F32)    # exp scale per key tile
        dqc_all = perm.tile([128, NT], F32)
        dqact_all = perm.tile([128, NT], F32)
        ap0 = dqs_in[:]
        nc.gpsimd.dma_start(out=dq_b[:], in_=bass.AP(
            tensor=ap0.tensor, offset=ap0.offset, ap=[[0, 128]] + list(ap0.ap)))

        nwpool = tc.alloc_tile_pool(name="nw", bufs=1)
        if not skip_anw:
            anw_b = nwpool.tile([128, H], F32, tag="anw")
            ap0 = anw_in[:]
            nc.gpsimd.dma_start(out=anw_b[:], in_=bass.AP(
                tensor=ap0.tensor, offset=ap0.offset, ap=[[0, 128]] + list(ap0.ap)))
        if not skip_fnw:
            fnw_b = nwpool.tile([128, H], F32, tag="fnw")
            ap0 = fnw_in[:]
            nc.gpsimd.dma_start(out=fnw_b[:], in_=bass.AP(
                tensor=ap0.tensor, offset=ap0.offset, ap=[[0, 128]] + list(ap0.ap)))

        # rope tables (freed after attention)
        csA = tc.alloc_tile_pool(name="csA", bufs=1)
        cos_st = csA.tile([128, NT, 4, 64], F32, tag="cosst")
        sin_st = csA.tile([128, NT, 4, 64], F32, tag="sinst")
        with tc.tile_pool(name="p0", bufs=2) as p0:
            cs = p0.tile([128, NT, 64], F32, tag="cscmp")
            nc.scalar.dma_start(cs[:], cos_in.rearrange("t p f -> p t f"))
            for c in range(4):
                nc.vector.tensor_copy(cos_st[:, :, c, :], cs[:])
            sn = p0.tile([128, NT, 64], F32, tag="cscmp")
            nc.scalar.dma_start(sn[:], sin_in.rearrange("t p f -> p t f"))
            for c in range(4):
                nc.vector.tensor_copy(sin_st[:, :, c, :], sn[:])

        ctxA = tc.alloc_tile_pool(name="ctxA", bufs=1, side="right")
        ctx_t = []
        for t in range(NT):
            c_tile = ctxA.tile([128, NH, 128], BF16, tag=f"ctx{t}")
            ctx_t.append(c_tile)

        # ------------ N1: attn rmsnorm + act-quant + transpose ---------------
        slotA = tc.alloc_tile_pool(name="slotA", bufs=1)
        xqT_t = []
        for t in range(NT):
            x_tile = slotA.tile([128, 16, 128], BF16, tag=f"xqT{t}")
            xqT_t.append(x_tile)

        # prefetch the first two qkv strips before the norm chain
        qkarena = tc.alloc_tile_pool(name="qkarena", bufs=2)
        wqpool = tc.alloc_tile_pool(name="wqpool", bufs=2, side="right")
        wst_tiles = {}

        def load_qkv_strip(j):
            w_tile = wqpool.tile([128, 16, 512], BF16, tag="wst")
            nc.scalar.dma_start(w_tile[:], wqkv_in[j].rearrange("k p f -> p k f"))
            wst_tiles[j] = w_tile

        load_qkv_strip(0)
        load_qkv_strip(1)

        with tc.tile_pool(name="npool", bufs=2) as npool:
            for t in range(NT):
                xt = npool.tile([128, H], F32, tag="xt")
                nc.scalar.dma_start(xt[:], x_in[t * 128:(t + 1) * 128, :])
                ssq = npool.tile([128, 1], F32, tag="ssq")
                junk = npool.tile([128, H], BF16, tag="njunk", bufs=1)
                nc.scalar.activation(junk[:], xt[:], ACTF.Square, accum_out=ssq[:])
                msq = npool.tile([128, 1], F32, tag="msq")
                nc.vector.tensor_scalar(msq[:], ssq[:], 1.0 / H, EPS, OP.mult, OP.add)
                sd = npool.tile([128, 1], F32, tag="sd")
                nc.scalar.activation(sd[:], msq[:], ACTF.Sqrt)
                rstd = npool.tile([128, 1], F32, tag="rstd")
                nc.vector.reciprocal(rstd[:], sd[:])
                if skip_anw:
                    amax = npool.tile([128, 1], F32, tag="amax")
                    nc.vector.tensor_reduce(amax[:], xt[:], AX.X, OP.max,
                                            apply_absolute_value=True)
                    amc = npool.tile([128, 1], F32, tag="amc")
                    nc.vector.tensor_scalar(amc[:], amax[:], rstd[:], 1e-5,
                                            OP.mult, OP.max)
                    rec = npool.tile([128, 1], F32, tag="rec")
                    nc.vector.reciprocal(rec[:], amc[:])
                    smul = npool.tile([128, 1], F32, tag="smul")
                    nc.vector.tensor_scalar(smul[:], rec[:], rstd[:], 127.0,
                                            OP.mult, OP.mult)
                    src_ap = xt[:]
                else:
                    h_t = npool.tile([128, H], F32, tag="h_t")
                    nc.vector.tensor_scalar_mul(h_t[:], xt[:], rstd[:])
                    nc.vector.tensor_tensor(h_t[:], h_t[:], anw_b[:], OP.mult)
                    amax = npool.tile([128, 1], F32, tag="amax")
                    nc.vector.tensor_reduce(amax[:], h_t[:], AX.X, OP.max,
                                            apply_absolute_value=True)
                    amc = npool.tile([128, 1], F32, tag="amc")
                    nc.vector.tensor_scalar_max(amc[:], amax[:], 1e-5)
                    rec = npool.tile([128, 1], F32, tag="rec")
                    nc.vector.reciprocal(rec[:], amc[:])
                    smul = npool.tile([128, 1], F32, tag="smul")
                    nc.vector.tensor_scalar_mul(smul[:], rec[:], 127.0)
                    src_ap = h_t[:]
                nc.vector.tensor_scalar(dqa_all[:, t:t + 1], amc[:], dq_b[:, 0:1],
                                        None, OP.mult)
                nc.vector.tensor_scalar_mul(e_scale[:, t:t + 1],
                                            dqa_all[:, t:t + 1], SCALE_QK)
                mg = npool.tile([128, H], F32, tag="mg")
                nc.scalar.activation(mg[:], src_ap, ACTF.Identity,
                                     bias=magic_ap, scale=smul[:])
                xq = npool.tile([128, H], BF16, tag="xq")
                nc.vector.tensor_scalar_sub(xq[:], mg[:], float(MAGIC))
                nc.sync.dma_start_transpose(xqT_t[t][:], xq[:])

        # ------------ QKV + rope + block attention, per 4-head group ---------
        with tc.tile_pool(name="rpool", bufs=2) as rpool, \
             tc.tile_pool(name="apool", bufs=2) as apool, \
             tc.tile_pool(name="qkv_psum", bufs=3, space="PSUM") as psum_mm, \
             tc.tile_pool(name="at_psum", bufs=2, space="PSUM") as psum_at, \
             tc.tile_pool(name="ctx_psum", bufs=2, space="PSUM") as psum_cx:
            for g in range(4):
                qTg = qkarena.tile([128, 4, NT, 128], BF16, tag="qTg")
                kTg = qkarena.tile([128, 4, NT, 128], BF16, tag="kTg")
                vg = qkarena.tile([128, NT, 4, 132], BF16, tag="vg")
                nc.vector.memset(vg[:, :, :, 128:129], 1.0)
                for sub in range(3):     # q, k, v strips of this head group
                    j = 3 * g + sub
                    if j not in wst_tiles:
                        load_qkv_strip(j)
                    wst = wst_tiles[j]
                    for t in range(NT):
                        ps = psum_mm.tile([128, 512], F32, tag="ps_qkv")
                        for kk in range(16):
                            nc.tensor.matmul(ps[:], xqT_t[t][:, kk, :],
                                             wst[:, kk, :],
                                             start=(kk == 0), stop=(kk == 15))
                        dqa_ap = dqa_all[:, t:t + 1]
                        if sub == 2:     # V: dequant into the augmented tile
                            nc.vector.tensor_scalar_mul(
                                vg[:, t, :, 0:128],
                                ps[:].rearrange("p (c f) -> p c f", c=4), dqa_ap)
                        else:
                            cosd = cos_st[:, t, :, :]
                            sind = sin_st[:, t, :, :]
                            rot = rpool.tile([128, 4, 128], BF16, tag="rot")
                            if sub == 0:    # Q: dequant then rope (gpsimd ok)
                                qsc = rpool.tile([128, 512], F32, tag="qsc")
                                nc.vector.tensor_scalar_mul(qsc[:], ps[:], dqa_ap)
                                q3 = qsc[:].rearrange("p (c f) -> p c f", c=4)
                                p1, p2 = q3[:, :, 0:64], q3[:, :, 64:128]
                                t1 = rpool.tile([128, 4, 64], F32, tag="rt1")
                                t2 = rpool.tile([128, 4, 64], F32, tag="rt2")
                                nc.vector.tensor_tensor(t1[:], p1, cosd, OP.mult)
                                nc.gpsimd.tensor_tensor(t2[:], p2, sind, OP.mult)
                                nc.vector.tensor_tensor(rot[:, :, 0:64], t1[:],
                                                        t2[:], OP.subtract)
                                nc.vector.tensor_tensor(t1[:], p2, cosd, OP.mult)
                                nc.gpsimd.tensor_tensor(t2[:], p1, sind, OP.mult)
                                nc.vector.tensor_tensor(rot[:, :, 64:128], t1[:],
                                                        t2[:], OP.add)
                            else:           # K: rope raw ints from PSUM (DVE only;
                                            # dequant folds into the exp scale)
                                k3 = ps[:].rearrange("p (c f) -> p c f", c=4)
                                p1, p2 = k3[:, :, 0:64], k3[:, :, 64:128]
                                t1 = rpool.tile([128, 4, 64], F32, tag="rt1")
                                t2 = rpool.tile([128, 4, 64], F32, tag="rt2")
                                nc.vector.tensor_tensor(t1[:], p1, cosd, OP.mult)
                                nc.vector.tensor_tensor(t2[:], p2, sind, OP.mult)
                                nc.vector.tensor_tensor(rot[:, :, 0:64], t1[:],
                                                        t2[:], OP.subtract)
                                nc.vector.tensor_tensor(t1[:], p2, cosd, OP.mult)
                                nc.vector.tensor_tensor(t2[:], p1, sind, OP.mult)
                                nc.vector.tensor_tensor(rot[:, :, 64:128], t1[:],
                                                        t2[:], OP.add)
                            dst = qTg if sub == 0 else kTg
                            nc.sync.dma_start_transpose(
                                dst[:, :, t, :],
                                rot[:].rearrange("p c f -> p (c f)"))
                # block attention for heads 4g..4g+4
                for blk in range(2):
                    for hh in range(4):
                        expT = [None] * 4
                        for kt in range(4):
                            qn = 512 - kt * 128
                            pss = psum_at.tile([128, 512], F32, tag="ps_sc")
                            nc.tensor.matmul(
                                pss[:, 0:qn],
                                kTg[:, hh, blk * 4 + kt, :],
                                qTg[:, hh, blk * 4:(blk + 1) * 4, :]
                                .rearrange("p c f -> p (c f)")[:, kt * 128:512],
                                start=True, stop=True)
                            ex = apool.tile([128, 512], BF16, tag=f"expT{kt}")
                            nc.scalar.activation(
                                ex[:, 0:qn], pss[:, 0:qn], ACTF.Exp,
                                scale=e_scale[:, blk * 4 + kt:blk * 4 + kt + 1])
                            nc.gpsimd.affine_select(
                                out=ex[:, 0:128], in_=ex[:, 0:128],
                                compare_op=OP.is_ge, fill=0.0,
                                base=0, pattern=[[1, 128]], channel_multiplier=-1)
                            expT[kt] = ex
                        for qt in range(4):
                            psc = psum_cx.tile([128, 132], F32, tag="ps_ctx")
                            for kt in range(qt + 1):
                                nc.tensor.matmul(
                                    psc[:, 0:129],
                                    expT[kt][:, (qt - kt) * 128:(qt - kt) * 128 + 128],
                                    vg[:, blk * 4 + kt, hh, 0:129],
                                    start=(kt == 0), stop=(kt == qt))
                            rl = apool.tile([128, 1], F32, tag="rl")
                            nc.vector.reciprocal(rl[:], psc[:, 128:129])
                            nc.vector.tensor_scalar_mul(
                                ctx_t[blk * 4 + qt][:, 4 * g + hh, :],
                                psc[:, 0:128], rl[:])
        wqpool.release()
        qkarena.release()
        slotA.release()
        csA.release()

        # o_proj weights prefetch during ctx quant
        wopool = tc.alloc_tile_pool(name="wopool", bufs=1)
        wot = wopool.tile([128, 16, 4, 512], BF16, tag="wot")
        for c4 in range(4):
            nc.scalar.dma_start(wot[:, :, c4, :],
                                wo_in[c4].rearrange("k p f -> p k f"))

        # ------------ ctx act-quant + transpose ------------------------------
        ctxqA = tc.alloc_tile_pool(name="ctxqA", bufs=1)
        ctxqT_t = []
        for t in range(NT):
            cq_tile = ctxqA.tile([128, 16, 128], BF16, tag=f"ctxqT{t}")
            ctxqT_t.append(cq_tile)
        with tc.tile_pool(name="cqpool", bufs=2) as cqpool:
            for t in range(NT):
                src = ctx_t[t][:].rearrange("p c f -> p (c f)")
                amax = cqpool.tile([128, 1], F32, tag="camax")
                nc.vector.tensor_reduce(amax[:], src, AX.X, OP.max,
                                        apply_absolute_value=True)
                amc = cqpool.tile([128, 1], F32, tag="camc")
                nc.vector.tensor_scalar_max(amc[:], amax[:], 1e-5)
                rec = cqpool.tile([128, 1], F32, tag="crec")
                nc.vector.reciprocal(rec[:], amc[:])
                s_c = cqpool.tile([128, 1], F32, tag="cs")
                nc.vector.tensor_scalar_mul(s_c[:], rec[:], 127.0)
                nc.vector.tensor_scalar(dqc_all[:, t:t + 1], amc[:], dq_b[:, 1:2],
                                        None, OP.mult)
                mg = cqpool.tile([128, H], F32, tag="cmg")
                nc.scalar.activation(mg[:], src, ACTF.Identity,
                                     bias=magic_ap, scale=s_c[:])
                cq = cqpool.tile([128, H], BF16, tag="cq")
                nc.vector.tensor_scalar_sub(cq[:], mg[:], float(MAGIC))
                nc.sync.dma_start_transpose(ctxqT_t[t][:], cq[:])
        ctxA.release()

        # down-proj residual staging lives on the right for the whole ffn
        dorpool = tc.alloc_tile_pool(name="dorpool", bufs=2, side="right")

        # ------------ o_proj + residual + ffn rmsnorm (fused per tile) -------
        hnA = tc.alloc_tile_pool(name="hnA", bufs=1, side="right")
        hnT_t = []
        for t in range(NT):
            h_tile = hnA.tile([128, 16, 128], F16, tag=f"hnT{t}")
            hnT_t.append(h_tile)
        with tc.tile_pool(name="opool", bufs=3) as opool, \
             tc.tile_pool(name="o_psum", bufs=3, space="PSUM") as psum_o:
            for t in range(NT):
                x1t = opool.tile([128, H], F32, tag="x1t")
                for nn in range(4):
                    ps = psum_o.tile([128, 512], F32, tag="ps_o")
                    for kk in range(16):
                        nc.tensor.matmul(ps[:], ctxqT_t[t][:, kk, :],
                                         wot[:, kk, nn, :],
                                         start=(kk == 0), stop=(kk == 15))
                    xs = opool.tile([128, 512], F32, tag="xs")
                    nc.scalar.dma_start(xs[:], x_in[t * 128:(t + 1) * 128,
                                                    nn * 512:(nn + 1) * 512])
                    tmp = opool.tile([128, 512], F32, tag="o_tmp")
                    nc.vector.tensor_scalar_mul(tmp[:], ps[:], dqc_all[:, t:t + 1])
                    nc.gpsimd.tensor_tensor(x1t[:, nn * 512:(nn + 1) * 512],
                                            tmp[:], xs[:], OP.add)
                nc.sync.dma_start(x1_d[t, :, :], x1t[:])
                # ffn rmsnorm on the in-SBUF x1 tile
                ssq = opool.tile([128, 1], F32, tag="ssq2")
                junk = opool.tile([128, H], BF16, tag="njunk2")
                nc.scalar.activation(junk[:], x1t[:], ACTF.Square, accum_out=ssq[:])
                msq = opool.tile([128, 1], F32, tag="msq2")
                nc.vector.tensor_scalar(msq[:], ssq[:], 1.0 / H, EPS, OP.mult, OP.add)
                sd = opool.tile([128, 1], F32, tag="sd2")
                nc.scalar.activation(sd[:], msq[:], ACTF.Sqrt)
                rstd = opool.tile([128, 1], F32, tag="rstd2")
                nc.vector.reciprocal(rstd[:], sd[:])
                hn16 = opool.tile([128, H], F16, tag="hn16")
                if skip_fnw:
                    nc.scalar.activation(hn16[:], x1t[:], ACTF.Identity,
                                         scale=rstd[:])
                else:
                    hn = opool.tile([128, H], F32, tag="hnf")
                    nc.vector.tensor_scalar_mul(hn[:], x1t[:], rstd[:])
                    nc.vector.tensor_tensor(hn16[:], hn[:], fnw_b[:], OP.mult)
                nc.sync.dma_start_transpose(hnT_t[t][:], hn16[:])
        ctxqA.release()
        wopool.release()

        # actqT arena + ffn_down weight prefetch start before ffn_up runs
        aqA = tc.alloc_tile_pool(name="aqA", bufs=1)
        actqT_t = []
        for t in range(NT):
            aq_tile = aqA.tile([128, 32, 128], BF16, tag=f"actqT{t}")
            actqT_t.append(aq_tile)
        dpool = tc.alloc_tile_pool(name="dpool", bufs=4)
        wdt_tiles = {}

        def load_dn_strip(nn):
            qs = []
            for q4 in range(4):
                tq = dpool.tile([128, 8, 512], BF16, tag="wdtQ")
                nc.scalar.dma_start(
                    tq[:], wdn_in[nn, q4 * 8:(q4 + 1) * 8]
                    .rearrange("k p f -> p k f"))
                qs.append(tq)
            wdt_tiles[nn] = qs

        load_dn_strip(0)

        # ------------ ffn up (fp16, unpaired gate/val) -> act_d --------------
        svpool = tc.alloc_tile_pool(name="svpool", bufs=1, side="right")
        sg_t = []
        for t in range(NT):
            sgt = svpool.tile([128, 512], BF16, tag=f"sg{t}")
            sg_t.append(sgt)
        with tc.tile_pool(name="upool", bufs=2, side="right") as upool, \
             tc.tile_pool(name="fpool", bufs=2) as fpool, \
             tc.tile_pool(name="aqpool", bufs=2) as aqpool, \
             tc.tile_pool(name="up_psum", bufs=3, space="PSUM") as psum_up, \
             tc.tile_pool(name="dn_psum", bufs=3, space="PSUM") as psum_dn:
            for j in range(16):          # gate0, val0, gate1, val1, ...
                wu = upool.tile([128, 16, 512], F16, tag="wup")
                nc.scalar.dma_start(wu[:], wup_in[j].rearrange("k p f -> p k f"))
                for t in range(NT):
                    ps = psum_up.tile([128, 512], F32, tag="ps_up")
                    for kk in range(16):
                        nc.tensor.matmul(ps[:], hnT_t[t][:, kk, :], wu[:, kk, :],
                                         start=(kk == 0), stop=(kk == 15))
                    if j % 2 == 0:       # gate strip: stage silu(gate)
                        sgm = fpool.tile([128, 512], F32, tag="sgm")
                        nc.scalar.activation(sgm[:], ps[:], ACTF.Sigmoid)
                        nc.vector.tensor_tensor(sg_t[t][:], sgm[:], ps[:], OP.mult)
                    else:                # val strip: act = silu(gate)*val
                        av = fpool.tile([128, 512], F16, tag="av")
                        nc.vector.tensor_tensor(av[:], sg_t[t][:], ps[:], OP.mult)
                        i = j // 2
                        nc.sync.dma_start(act_d[t, :, i * 512:(i + 1) * 512],
                                          av[:])

            # ---- act quant + transpose (overlaps the ffn_up tail) ----
            for t in range(NT):
                at_h = []
                amax2 = aqpool.tile([128, 2], F32, tag="aamax2")
                for half in range(2):
                    ah = aqpool.tile([128, 2048], F16, tag="at_h")
                    nc.scalar.dma_start(ah[:], act_d[t, :, half * 2048:
                                                     (half + 1) * 2048])
                    nc.vector.tensor_reduce(amax2[:, half:half + 1], ah[:],
                                            AX.X, OP.max,
                                            apply_absolute_value=True)
                    at_h.append(ah)
                amax = aqpool.tile([128, 1], F32, tag="aamax")
                nc.vector.tensor_reduce(amax[:], amax2[:], AX.X, OP.max)
                amc = aqpool.tile([128, 1], F32, tag="aamc")
                nc.vector.tensor_scalar_max(amc[:], amax[:], 1e-5)
                rec = aqpool.tile([128, 1], F32, tag="arec")
                nc.vector.reciprocal(rec[:], amc[:])
                s_a = aqpool.tile([128, 1], F32, tag="as")
                nc.vector.tensor_scalar_mul(s_a[:], rec[:], 127.0)
                nc.vector.tensor_scalar(dqact_all[:, t:t + 1], amc[:],
                                        dq_b[:, 2:3], None, OP.mult)
                for half in range(2):
                    mg = aqpool.tile([128, 2048], F32, tag="amg", bufs=1)
                    nc.scalar.activation(mg[:], at_h[half][:], ACTF.Identity,
                                         bias=magic_ap, scale=s_a[:])
                    aq = aqpool.tile([128, 2048], BF16, tag="aq", bufs=1)
                    nc.vector.tensor_scalar_sub(aq[:], mg[:], float(MAGIC))
                    nc.sync.dma_start_transpose(
                        actqT_t[t][:, half * 16:(half + 1) * 16, :], aq[:])

            # ---- ffn down + residual -> out (overlaps act quant) ----
            for nn in range(4):
                if nn not in wdt_tiles:
                    load_dn_strip(nn)
                qs = wdt_tiles[nn]
                for t in range(NT):
                    ps = psum_dn.tile([128, 512], F32, tag="ps_dn")
                    for kk in range(32):
                        nc.tensor.matmul(ps[:], actqT_t[t][:, kk, :],
                                         qs[kk // 8][:, kk % 8, :],
                                         start=(kk == 0), stop=(kk == 31))
                    x1s = dorpool.tile([128, 512], F32, tag="x1s")
                    nc.scalar.dma_start(x1s[:], x1_d[t, :, nn * 512:(nn + 1) * 512])
                    tmp = dorpool.tile([128, 512], F32, tag="d_tmp")
                    nc.vector.tensor_scalar_mul(tmp[:], ps[:], dqact_all[:, t:t + 1])
                    ot = dorpool.tile([128, 512], F32, tag="ot")
                    nc.gpsimd.tensor_tensor(ot[:], tmp[:], x1s[:], OP.add)
                    nc.sync.dma_start(out_d[t * 128:(t + 1) * 128,
                                            nn * 512:(nn + 1) * 512], ot[:])
        svpool.release()
        hnA.release()
        dorpool.release()
        dpool.release()
        aqA.release()
        nwpool.release()
        perm.release()

    nc.compile()
    return nc


_NC_CACHE = {}


def _get_nc(skip_anw, skip_fnw):
    key = (skip_anw, skip_fnw)
    if key not in _NC_CACHE:
        _NC_CACHE[key] = build_program(skip_anw, skip_fnw)
    return _NC_CACHE[key]


def _ternarize(w):
    """Exact reference _weight_quant: per-tensor ternary + fp32 scale."""
    w = np.asarray(w, np.float32)
    m = np.float32(max(np.float32(np.mean(np.abs(w))), np.float32(1e-5)))
    scale = np.float32(1.0) / m
    t = np.clip(np.rint(w * scale), -1, 1)
    return t, np.float32(m)


def _host_inputs(x, attn_norm_w, ffn_norm_w, qkv_w, o_w, ffn_up_w, ffn_down_w):
    x = np.ascontiguousarray(np.asarray(x, np.float32))
    anw = np.ascontiguousarray(np.asarray(attn_norm_w, np.float32))
    fnw = np.ascontiguousarray(np.asarray(ffn_norm_w, np.float32))

    tq, m_qkv = _ternarize(np.asarray(qkv_w, np.float32).T)       # [2048, 6144]
    to, m_o = _ternarize(np.asarray(o_w, np.float32).T)           # [2048, 2048]
    td, m_dn = _ternarize(np.asarray(ffn_down_w, np.float32).T)   # [4096, 2048]

    wqkv3 = np.ascontiguousarray(
        tq.reshape(16, 128, 12, 512).transpose(2, 0, 1, 3)[QKV_ORDER]
    ).astype(ml_dtypes.bfloat16)
    wo3 = np.ascontiguousarray(
        to.reshape(16, 128, 4, 512).transpose(2, 0, 1, 3)
    ).astype(ml_dtypes.bfloat16)
    wdn3 = np.ascontiguousarray(
        td.reshape(32, 128, 4, 512).transpose(2, 0, 1, 3)
    ).astype(ml_dtypes.bfloat16)
    wup = np.asarray(ffn_up_w, np.float32).T                      # [2048, 8192]
    wup_h = np.ascontiguousarray(
        wup.reshape(16, 128, 16, 512).transpose(2, 0, 1, 3)[UP_ORDER]
    ).astype(np.float16)

    dqs = np.array([m_qkv / 127.0, m_o / 127.0, m_dn / 127.0, 0.0], np.float32)

    inv = 1.0 / (THETA ** (np.arange(0, HD, 2, dtype=np.float32) / HD))
    tpos = np.arange(S, dtype=np.float32)
    fr = np.outer(tpos, inv)                     # [S, 64]
    cosf = np.cos(fr).astype(np.float32)
    sinf = np.sin(fr).astype(np.float32)

    in_maps = []
    for c in range(NCORES):
        b = c // 4
        t0 = (c % 4) * R
        in_maps.append({
            "x_sh": np.ascontiguousarray(x[b, t0:t0 + R, :]),
            "cosb": np.ascontiguousarray(cosf[t0:t0 + R].reshape(NT, 128, 64)),
            "sinb": np.ascontiguousarray(sinf[t0:t0 + R].reshape(NT, 128, 64)),
            "attn_norm_w": anw, "ffn_norm_w": fnw, "dqs": dqs,
            "wqkv3": wqkv3, "wo3": wo3, "wup_h": wup_h, "wdn3": wdn3,
        })
    skip_anw = bool(np.all(anw == 1.0))
    skip_fnw = bool(np.all(fnw == 1.0))
    return in_maps, skip_anw, skip_fnw


def run(trace=False, **inputs):
    in_maps, skip_anw, skip_fnw = _host_inputs(**inputs)
    nc = _get_nc(skip_anw, skip_fnw)
    res = run_bass_kernel_spmd(nc, in_maps, list(range(NCORES)), trace=trace)
    out = np.empty((B, S, H), np.float32)
    for c in range(NCORES):
        b = c // 4
        t0 = (c % 4) * R
        out[b, t0:t0 + R, :] = res.results[c]["out_sh"]
    return out, res


def kernel(**inputs):
    out, _ = run(trace=False, **inputs)
    return out


# revision 21
# speedup vs baseline: 1.0852x; 1.0852x over previous
"""Trainium2 Bass kernel for nn_BlockAttentionResidual (block attention + BitNet MLP).

Sharding: sequence-block data parallelism. Block attention is independent per
512-token block, so each of the 8 cores owns 1024 contiguous tokens (2 blocks)
of one batch element and runs the ENTIRE layer on them with zero collectives.
  core c -> batch c//4, tokens [(c%4)*1024, (c%4+1)*1024)

All weight quantization happens on the host (ternary weights shipped as bf16,
exact), ffn_up ships as fp16 (~= f32r precision at half the DMA bytes); compact
rope tables; o_proj fused with the ffn rmsnorm; K-side dequant folded into the
softmax exp scale.  Activation tensors that cross phase boundaries are split
per 128-token tile so each consumer phase starts as soon as its own tile is
ready; pure loads issue on the scalar HWDGE queue so they never queue behind
compute-dependent transposes/stores on sync.

Numerics: bitlinear matmuls run on integer-valued bf16 operands (act-quant ints
in [-128,127] x ternary weights are exact in bf16; PSUM accumulates fp32);
attention in bf16 with fp32 accum; softmax skips max-subtraction (|score*scale|
~ 3); ffn_up in fp16 (10 mantissa bits = f32r).
"""

import numpy as np
import ml_dtypes

import concourse.bass as bass
import concourse.mybir as mybir
import concourse.tile as tile
from concourse import bacc
from concourse.bass_utils import run_bass_kernel_spmd

F32 = mybir.dt.float32
BF16 = mybir.dt.bfloat16
F16 = mybir.dt.float16
AX = mybir.AxisListType
OP = mybir.AluOpType
ACTF = mybir.ActivationFunctionType

# model dims
H = 2048
NH = 16
HD = 128
INTER = 4096        # reference INTER = 2*H; ffn_up rows = 2*INTER = 8192
EPS = 1e-5
THETA = 10000.0
B, S = 2, 4096
NCORES = 8
R = 1024            # tokens per core (= 2 attention blocks of 512)
NT = R // 128       # 8 token tiles per core
MAGIC = np.float32(1.5 * 2 ** 23)   # fp32 round-to-nearest-even magic
SCALE_QK = float(HD ** -0.5)

# strip order for the interleaved qkv stream: per head-group g the q, k, v
# strips (cols g*512 | 2048+g*512 | 4096+g*512 of w_qkv^T)
QKV_ORDER = [0, 4, 8, 1, 5, 9, 2, 6, 10, 3, 7, 11]
UP_ORDER = [0, 8, 1, 9, 2, 10, 3, 11, 4, 12, 5, 13, 6, 14, 7, 15]


def build_program(skip_anw, skip_fnw):
    nc = bacc.Bacc(None, target_bir_lowering=False)

    # ---- I/O ----
    x_in = nc.declare_dram_parameter("x_sh", [R, H], F32, isOutput=False)
    cos_in = nc.declare_dram_parameter("cosb", [NT, 128, 64], F32, isOutput=False)
    sin_in = nc.declare_dram_parameter("sinb", [NT, 128, 64], F32, isOutput=False)
    anw_in = nc.declare_dram_parameter("attn_norm_w", [H], F32, isOutput=False)
    fnw_in = nc.declare_dram_parameter("ffn_norm_w", [H], F32, isOutput=False)
    dqs_in = nc.declare_dram_parameter("dqs", [4], F32, isOutput=False)
    wqkv_in = nc.declare_dram_parameter("wqkv3", [12, 16, 128, 512], BF16, isOutput=False)
    wo_in = nc.declare_dram_parameter("wo3", [4, 16, 128, 512], BF16, isOutput=False)
    wup_in = nc.declare_dram_parameter("wup_h", [16, 16, 128, 512], F16, isOutput=False)
    wdn_in = nc.declare_dram_parameter("wdn3", [4, 32, 128, 512], BF16, isOutput=False)
    out_d = nc.declare_dram_parameter("out_sh", [R, H], F32, isOutput=True)

    # ---- internal DRAM scratch ----
    x1_d = nc.dram_tensor("x1_d", [NT, 128, H], F32)
    act_d = nc.dram_tensor("act_d", [NT, 128, INTER], F16)

    with tile.TileContext(nc) as tc:
        perm = tc.alloc_tile_pool(name="perm", bufs=1)
        magic_t = perm.tile([128, 1], F32)
        nc.vector.memset(magic_t[:], float(MAGIC))
        magic_ap = magic_t[:]
        dq_b = perm.tile([128, 4], F32)
        dqa_all = perm.tile([128, NT], F32)    # per-token dequant for q/v
        e_scale = perm.tile([128, NT], F32)    # exp scale per key tile
        dqc_all = perm.tile([128, NT], F32)
        dqact_all = perm.tile([128, NT], F32)
        ap0 = dqs_in[:]
        nc.gpsimd.dma_start(out=dq_b[:], in_=bass.AP(
            tensor=ap0.tensor, offset=ap0.offset, ap=[[0, 128]] + list(ap0.ap)))

        nwpool = tc.alloc_tile_pool(name="nw", bufs=1)
        if not skip_anw:
            anw_b = nwpool.tile([128, H], F32, tag="anw")
            ap0 = anw_in[:]
            nc.gpsimd.dma_start(out=anw_b[:], in_=bass.AP(
                tensor=ap0.tensor, offset=ap0.offset, ap=[[0, 128]] + list(ap0.ap)))
        if not skip_fnw:
            fnw_b = nwpool.tile([128, H], F32, tag="fnw")
            ap0 = fnw_in[:]
            nc.gpsimd.dma_start(out=fnw_b[:], in_=bass.AP(
                tensor=ap0.tensor, offset=ap0.offset, ap=[[0, 128]] + list(ap0.ap)))

        # rope tables, compact; rope ops broadcast over the 4-head dim via
        # stride-0 access patterns
        csA = tc.alloc_tile_pool(name="csA", bufs=1)
        cos_st = csA.tile([128, NT, 64], F32, tag="cosst")
        sin_st = csA.tile([128, NT, 64], F32, tag="sinst")
        nc.sync.dma_start(cos_st[:], cos_in.rearrange("t p f -> p t f"))
        nc.sync.dma_start(sin_st[:], sin_in.rearrange("t p f -> p t f"))

        def bcast4(tbl, t):
            a = tbl[:, t, :]
            return bass.AP(tensor=a.tensor, offset=a.offset,
                           ap=[list(a.ap[0]), [0, 4], list(a.ap[1])])

        # residual staging for ffn_down lives bottom-of-right-stack (dies last)
        dorpool = tc.alloc_tile_pool(name="dorpool", bufs=2, side="right")
        ctxA = tc.alloc_tile_pool(name="ctxA", bufs=1, side="right")
        ctx_t = []
        for t in range(NT):
            c_tile = ctxA.tile([128, NH, 128], BF16, tag=f"ctx{t}")
            ctx_t.append(c_tile)

        # ------------ N1: attn rmsnorm + act-quant + transpose ---------------
        slotA = tc.alloc_tile_pool(name="slotA", bufs=1)
        xqT_t = []
        for t in range(NT):
            x_tile = slotA.tile([128, 16, 128], BF16, tag=f"xqT{t}")
            xqT_t.append(x_tile)

        # prefetch the first two qkv strips before the norm chain
        qkarena = tc.alloc_tile_pool(name="qkarena", bufs=2)
        wqpool = tc.alloc_tile_pool(name="wqpool", bufs=2, side="right")
        wst_tiles = {}

        def load_qkv_strip(j):
            w_tile = wqpool.tile([128, 16, 512], BF16, tag="wst")
            nc.sync.dma_start(w_tile[:], wqkv_in[j].rearrange("k p f -> p k f"))
            wst_tiles[j] = w_tile

        load_qkv_strip(0)
        load_qkv_strip(1)

        with tc.tile_pool(name="npool", bufs=2) as npool:
            for t in range(NT):
                xt = npool.tile([128, H], F32, tag="xt")
                nc.sync.dma_start(xt[:], x_in[t * 128:(t + 1) * 128, :])
                ssq = npool.tile([128, 1], F32, tag="ssq")
                junk = npool.tile([128, H], BF16, tag="njunk", bufs=1)
                nc.scalar.activation(junk[:], xt[:], ACTF.Square, accum_out=ssq[:])
                msq = npool.tile([128, 1], F32, tag="msq")
                nc.vector.tensor_scalar(msq[:], ssq[:], 1.0 / H, EPS, OP.mult, OP.add)
                sd = npool.tile([128, 1], F32, tag="sd")
                nc.scalar.activation(sd[:], msq[:], ACTF.Sqrt)
                rstd = npool.tile([128, 1], F32, tag="rstd")
                nc.vector.reciprocal(rstd[:], sd[:])
                if skip_anw:
                    amax = npool.tile([128, 1], F32, tag="amax")
                    nc.vector.tensor_reduce(amax[:], xt[:], AX.X, OP.max,
                                            apply_absolute_value=True)
                    amc = npool.tile([128, 1], F32, tag="amc")
                    nc.vector.tensor_scalar(amc[:], amax[:], rstd[:], 1e-5,
                                            OP.mult, OP.max)
                    rec = npool.tile([128, 1], F32, tag="rec")
                    nc.vector.reciprocal(rec[:], amc[:])
                    smul = npool.tile([128, 1], F32, tag="smul")
                    nc.vector.tensor_scalar(smul[:], rec[:], rstd[:], 127.0,
                                            OP.mult, OP.mult)
                    src_ap = xt[:]
                else:
                    h_t = npool.tile([128, H], F32, tag="h_t")
                    nc.vector.tensor_scalar_mul(h_t[:], xt[:], rstd[:])
                    nc.vector.tensor_tensor(h_t[:], h_t[:], anw_b[:], OP.mult)
                    amax = npool.tile([128, 1], F32, tag="amax")
                    nc.vector.tensor_reduce(amax[:], h_t[:], AX.X, OP.max,
                                            apply_absolute_value=True)
                    amc = npool.tile([128, 1], F32, tag="amc")
                    nc.vector.tensor_scalar_max(amc[:], amax[:], 1e-5)
                    rec = npool.tile([128, 1], F32, tag="rec")
                    nc.vector.reciprocal(rec[:], amc[:])
                    smul = npool.tile([128, 1], F32, tag="smul")
                    nc.vector.tensor_scalar_mul(smul[:], rec[:], 127.0)
                    src_ap = h_t[:]
                nc.vector.tensor_scalar(dqa_all[:, t:t + 1], amc[:], dq_b[:, 0:1],
                                        None, OP.mult)
                nc.vector.tensor_scalar_mul(e_scale[:, t:t + 1],
                                            dqa_all[:, t:t + 1], SCALE_QK)
                mg = npool.tile([128, H], F32, tag="mg", bufs=1)
                nc.scalar.activation(mg[:], src_ap, ACTF.Identity,
                                     bias=magic_ap, scale=smul[:])
                xq = npool.tile([128, H], BF16, tag="xq")
                nc.vector.tensor_scalar_sub(xq[:], mg[:], float(MAGIC))
                nc.scalar.dma_start_transpose(xqT_t[t][:], xq[:])

        # ------------ QKV + rope + block attention, per 4-head group ---------
        with tc.tile_pool(name="rpool", bufs=2) as rpool, \
             tc.tile_pool(name="apool", bufs=2) as apool, \
             tc.tile_pool(name="qkv_psum", bufs=3, space="PSUM") as psum_mm, \
             tc.tile_pool(name="at_psum", bufs=2, space="PSUM") as psum_at, \
             tc.tile_pool(name="ctx_psum", bufs=2, space="PSUM") as psum_cx:
            for g in range(4):
                qTg = qkarena.tile([128, 4, NT, 128], BF16, tag="qTg")
                kTg = qkarena.tile([128, 4, NT, 128], BF16, tag="kTg")
                vg = qkarena.tile([128, NT, 4, 132], BF16, tag="vg")
                nc.vector.memset(vg[:, :, :, 128:129], 1.0)
                for sub in range(3):     # q, k, v strips of this head group
                    j = 3 * g + sub
                    if j not in wst_tiles:
                        load_qkv_strip(j)
                    wst = wst_tiles[j]
                    for t in range(NT):
                        ps = psum_mm.tile([128, 512], F32, tag="ps_qkv")
                        for kk in range(16):
                            nc.tensor.matmul(ps[:], xqT_t[t][:, kk, :],
                                             wst[:, kk, :],
                                             start=(kk == 0), stop=(kk == 15))
                        dqa_ap = dqa_all[:, t:t + 1]
                        if sub == 2:     # V: dequant into the augmented tile
                            nc.vector.tensor_scalar_mul(
                                vg[:, t, :, 0:128],
                                ps[:].rearrange("p (c f) -> p c f", c=4), dqa_ap)
                        else:
                            cosd = bcast4(cos_st, t)
                            sind = bcast4(sin_st, t)
                            rot = rpool.tile([128, 4, 128], BF16, tag="rot")
                            if sub == 0:    # Q: dequant then rope (gpsimd ok)
                                qsc = rpool.tile([128, 512], F32, tag="qsc")
                                nc.vector.tensor_scalar_mul(qsc[:], ps[:], dqa_ap)
                                q3 = qsc[:].rearrange("p (c f) -> p c f", c=4)
                                p1, p2 = q3[:, :, 0:64], q3[:, :, 64:128]
                                t1 = rpool.tile([128, 4, 64], F32, tag="rt1")
                                t2 = rpool.tile([128, 4, 64], F32, tag="rt2")
                                nc.vector.tensor_tensor(t1[:], p1, cosd, OP.mult)
                                nc.gpsimd.tensor_tensor(t2[:], p2, sind, OP.mult)
                                nc.vector.tensor_tensor(rot[:, :, 0:64], t1[:],
                                                        t2[:], OP.subtract)
                                nc.vector.tensor_tensor(t1[:], p2, cosd, OP.mult)
                                nc.gpsimd.tensor_tensor(t2[:], p1, sind, OP.mult)
                                nc.vector.tensor_tensor(rot[:, :, 64:128], t1[:],
                                                        t2[:], OP.add)
                            else:           # K: rope raw ints from PSUM (DVE only;
                                            # dequant folds into the exp scale)
                                k3 = ps[:].rearrange("p (c f) -> p c f", c=4)
                                p1, p2 = k3[:, :, 0:64], k3[:, :, 64:128]
                                t1 = rpool.tile([128, 4, 64], F32, tag="rt1")
                                t2 = rpool.tile([128, 4, 64], F32, tag="rt2")
                                nc.vector.tensor_tensor(t1[:], p1, cosd, OP.mult)
                                nc.vector.tensor_tensor(t2[:], p2, sind, OP.mult)
                                nc.vector.tensor_tensor(rot[:, :, 0:64], t1[:],
                                                        t2[:], OP.subtract)
                                nc.vector.tensor_tensor(t1[:], p2, cosd, OP.mult)
                                nc.vector.tensor_tensor(t2[:], p1, sind, OP.mult)
                                nc.vector.tensor_tensor(rot[:, :, 64:128], t1[:],
                                                        t2[:], OP.add)
                            dst = qTg if sub == 0 else kTg
                            nc.scalar.dma_start_transpose(
                                dst[:, :, t, :],
                                rot[:].rearrange("p c f -> p (c f)"))
                # block attention for heads 4g..4g+4
                for blk in range(2):
                    for hh in range(4):
                        expT = [None] * 4
                        for kt in range(4):
                            qn = 512 - kt * 128
                            pss = psum_at.tile([128, 512], F32, tag="ps_sc")
                            nc.tensor.matmul(
                                pss[:, 0:qn],
                                kTg[:, hh, blk * 4 + kt, :],
                                qTg[:, hh, blk * 4:(blk + 1) * 4, :]
                                .rearrange("p c f -> p (c f)")[:, kt * 128:512],
                                start=True, stop=True)
                            ex = apool.tile([128, 512], BF16, tag=f"expT{kt}")
                            nc.scalar.activation(
                                ex[:, 0:qn], pss[:, 0:qn], ACTF.Exp,
                                scale=e_scale[:, blk * 4 + kt:blk * 4 + kt + 1])
                            nc.gpsimd.affine_select(
                                out=ex[:, 0:128], in_=ex[:, 0:128],
                                compare_op=OP.is_ge, fill=0.0,
                                base=0, pattern=[[1, 128]], channel_multiplier=-1)
                            expT[kt] = ex
                        for qt in range(4):
                            psc = psum_cx.tile([128, 132], F32, tag="ps_ctx")
                            for kt in range(qt + 1):
                                nc.tensor.matmul(
                                    psc[:, 0:129],
                                    expT[kt][:, (qt - kt) * 128:(qt - kt) * 128 + 128],
                                    vg[:, blk * 4 + kt, hh, 0:129],
                                    start=(kt == 0), stop=(kt == qt))
                            rl = apool.tile([128, 1], F32, tag="rl")
                            nc.vector.reciprocal(rl[:], psc[:, 128:129])
                            nc.vector.tensor_scalar_mul(
                                ctx_t[blk * 4 + qt][:, 4 * g + hh, :],
                                psc[:, 0:128], rl[:])
        wqpool.release()
        qkarena.release()
        slotA.release()
        csA.release()

        # o_proj weights prefetch during g3 attention / ctx quant
        wopool = tc.alloc_tile_pool(name="wopool", bufs=1, side="right")
        wot = wopool.tile([128, 16, 4, 512], BF16, tag="wot")
        for c4 in range(4):
            nc.sync.dma_start(wot[:, :, c4, :],
                              wo_in[c4].rearrange("k p f -> p k f"))
        hnA = tc.alloc_tile_pool(name="hnA", bufs=1)
        hnT_t = []
        for t in range(NT):
            h_tile = hnA.tile([128, 16, 128], F16, tag=f"hnT{t}")
            hnT_t.append(h_tile)

        # ------------ ctx act-quant + transpose ------------------------------
        ctxqA = tc.alloc_tile_pool(name="ctxqA", bufs=1)
        ctxqT_t = []
        for t in range(NT):
            cq_tile = ctxqA.tile([128, 16, 128], BF16, tag=f"ctxqT{t}")
            ctxqT_t.append(cq_tile)
        with tc.tile_pool(name="cqpool", bufs=2) as cqpool:
            for t in range(NT):
                src = ctx_t[t][:].rearrange("p c f -> p (c f)")
                amax = cqpool.tile([128, 1], F32, tag="camax")
                nc.vector.tensor_reduce(amax[:], src, AX.X, OP.max,
                                        apply_absolute_value=True)
                amc = cqpool.tile([128, 1], F32, tag="camc")
                nc.vector.tensor_scalar_max(amc[:], amax[:], 1e-5)
                rec = cqpool.tile([128, 1], F32, tag="crec")
                nc.vector.reciprocal(rec[:], amc[:])
                s_c = cqpool.tile([128, 1], F32, tag="cs")
                nc.vector.tensor_scalar_mul(s_c[:], rec[:], 127.0)
                nc.vector.tensor_scalar(dqc_all[:, t:t + 1], amc[:], dq_b[:, 1:2],
                                        None, OP.mult)
                mg = cqpool.tile([128, H], F32, tag="cmg")
                nc.scalar.activation(mg[:], src, ACTF.Identity,
                                     bias=magic_ap, scale=s_c[:])
                cq = cqpool.tile([128, H], BF16, tag="cq")
                nc.vector.tensor_scalar_sub(cq[:], mg[:], float(MAGIC))
                nc.scalar.dma_start_transpose(ctxqT_t[t][:], cq[:])

        # ------------ o_proj + residual + ffn rmsnorm (fused per tile) -------
        with tc.tile_pool(name="opool", bufs=3) as opool, \
             tc.tile_pool(name="o_psum", bufs=3, space="PSUM") as psum_o:
            for t in range(NT):
                x1t = opool.tile([128, H], F32, tag="x1t", bufs=2)
                for nn in range(4):
                    ps = psum_o.tile([128, 512], F32, tag="ps_o")
                    for kk in range(16):
                        nc.tensor.matmul(ps[:], ctxqT_t[t][:, kk, :],
                                         wot[:, kk, nn, :],
                                         start=(kk == 0), stop=(kk == 15))
                    xs = opool.tile([128, 512], F32, tag="xs", bufs=2)
                    nc.sync.dma_start(xs[:], x_in[t * 128:(t + 1) * 128,
                                                    nn * 512:(nn + 1) * 512])
                    tmp = opool.tile([128, 512], F32, tag="o_tmp", bufs=1)
                    nc.vector.tensor_scalar_mul(tmp[:], ps[:], dqc_all[:, t:t + 1])
                    nc.gpsimd.tensor_tensor(x1t[:, nn * 512:(nn + 1) * 512],
                                            tmp[:], xs[:], OP.add)
                nc.scalar.dma_start(x1_d[t, :, :], x1t[:])
                # ffn rmsnorm on the in-SBUF x1 tile
                ssq = opool.tile([128, 1], F32, tag="ssq2")
                junk = opool.tile([128, H], BF16, tag="njunk2", bufs=1)
                nc.scalar.activation(junk[:], x1t[:], ACTF.Square, accum_out=ssq[:])
                msq = opool.tile([128, 1], F32, tag="msq2")
                nc.vector.tensor_scalar(msq[:], ssq[:], 1.0 / H, EPS, OP.mult, OP.add)
                sd = opool.tile([128, 1], F32, tag="sd2")
                nc.scalar.activation(sd[:], msq[:], ACTF.Sqrt)
                rstd = opool.tile([128, 1], F32, tag="rstd2")
                nc.vector.reciprocal(rstd[:], sd[:])
                hn16 = opool.tile([128, H], F16, tag="hn16", bufs=2)
                if skip_fnw:
                    nc.scalar.activation(hn16[:], x1t[:], ACTF.Identity,
                                         scale=rstd[:])
                else:
                    hn = opool.tile([128, H], F32, tag="hnf")
                    nc.vector.tensor_scalar_mul(hn[:], x1t[:], rstd[:])
                    nc.vector.tensor_tensor(hn16[:], hn[:], fnw_b[:], OP.mult)
                nc.scalar.dma_start_transpose(hnT_t[t][:], hn16[:])
        ctxqA.release()
        wopool.release()
        ctxA.release()

        # actqT arena + ffn_down weight prefetch start before ffn_up runs
        aqA = tc.alloc_tile_pool(name="aqA", bufs=1)
        actqT_t = []
        for t in range(NT):
            aq_tile = aqA.tile([128, 32, 128], BF16, tag=f"actqT{t}")
            actqT_t.append(aq_tile)
        dpool = tc.alloc_tile_pool(name="dpool", bufs=4)
        wdt_tiles = {}

        def load_dn_strip(nn):
            qs = []
            for q4 in range(4):
                tq = dpool.tile([128, 8, 512], BF16, tag="wdtQ")
                nc.sync.dma_start(
                    tq[:], wdn_in[nn, q4 * 8:(q4 + 1) * 8]
                    .rearrange("k p f -> p k f"))
                qs.append(tq)
            wdt_tiles[nn] = qs

        load_dn_strip(0)

        # ------------ ffn up (fp16, unpaired gate/val) -> act_d --------------
        svpool = tc.alloc_tile_pool(name="svpool", bufs=1, side="right")
        sg_t = []
        for t in range(NT):
            sgt = svpool.tile([128, 512], BF16, tag=f"sg{t}")
            sg_t.append(sgt)
        with tc.tile_pool(name="upool", bufs=2, side="right") as upool, \
             tc.tile_pool(name="fpool", bufs=2) as fpool, \
             tc.tile_pool(name="aqpool", bufs=2) as aqpool, \
             tc.tile_pool(name="up_psum", bufs=3, space="PSUM") as psum_up, \
             tc.tile_pool(name="dn_psum", bufs=3, space="PSUM") as psum_dn:
            for j in range(16):          # gate0, val0, gate1, val1, ...
                wu = upool.tile([128, 16, 512], F16, tag="wup")
                nc.sync.dma_start(wu[:], wup_in[j].rearrange("k p f -> p k f"))
                for t in range(NT):
                    ps = psum_up.tile([128, 512], F32, tag="ps_up")
                    for kk in range(16):
                        nc.tensor.matmul(ps[:], hnT_t[t][:, kk, :], wu[:, kk, :],
                                         start=(kk == 0), stop=(kk == 15))
                    if j % 2 == 0:       # gate strip: stage silu(gate)
                        sgm = fpool.tile([128, 512], F32, tag="sgm")
                        nc.scalar.activation(sgm[:], ps[:], ACTF.Sigmoid)
                        nc.vector.tensor_tensor(sg_t[t][:], sgm[:], ps[:], OP.mult)
                    else:                # val strip: act = silu(gate)*val
                        av = fpool.tile([128, 512], F16, tag="av")
                        nc.vector.tensor_tensor(av[:], sg_t[t][:], ps[:], OP.mult)
                        i = j // 2
                        nc.scalar.dma_start(act_d[t, :, i * 512:(i + 1) * 512],
                                            av[:])

            # ---- act quant + transpose (overlaps the ffn_up tail) ----
            for t in range(NT):
                at_h = []
                amax2 = aqpool.tile([128, 2], F32, tag="aamax2")
                for half in range(2):
                    ah = aqpool.tile([128, 2048], F16, tag="at_h")
                    nc.sync.dma_start(ah[:], act_d[t, :, half * 2048:
                                                     (half + 1) * 2048])
                    nc.vector.tensor_reduce(amax2[:, half:half + 1], ah[:],
                                            AX.X, OP.max,
                                            apply_absolute_value=True)
                    at_h.append(ah)
                amax = aqpool.tile([128, 1], F32, tag="aamax")
                nc.vector.tensor_reduce(amax[:], amax2[:], AX.X, OP.max)
                amc = aqpool.tile([128, 1], F32, tag="aamc")
                nc.vector.tensor_scalar_max(amc[:], amax[:], 1e-5)
                rec = aqpool.tile([128, 1], F32, tag="arec")
                nc.vector.reciprocal(rec[:], amc[:])
                s_a = aqpool.tile([128, 1], F32, tag="as")
                nc.vector.tensor_scalar_mul(s_a[:], rec[:], 127.0)
                nc.vector.tensor_scalar(dqact_all[:, t:t + 1], amc[:],
                                        dq_b[:, 2:3], None, OP.mult)
                for half in range(2):
                    mg = aqpool.tile([128, 2048], F32, tag="amg", bufs=1)
                    nc.scalar.activation(mg[:], at_h[half][:], ACTF.Identity,
                                         bias=magic_ap, scale=s_a[:])
                    aq = aqpool.tile([128, 2048], BF16, tag="aq", bufs=1)
                    nc.vector.tensor_scalar_sub(aq[:], mg[:], float(MAGIC))
                    nc.scalar.dma_start_transpose(
                        actqT_t[t][:, half * 16:(half + 1) * 16, :], aq[:])

            # ---- ffn down + residual -> out (overlaps act quant) ----
            for nn in range(4):
                if nn not in wdt_tiles:
                    load_dn_strip(nn)
                qs = wdt_tiles[nn]
                for t in range(NT):
                    ps = psum_dn.tile([128, 512], F32, tag="ps_dn")
                    for kk in range(32):
                        nc.tensor.matmul(ps[:], actqT_t[t][:, kk, :],
                                         qs[kk // 8][:, kk % 8, :],
                                         start=(kk == 0), stop=(kk == 31))
                    x1s = dorpool.tile([128, 512], F32, tag="x1s")
                    nc.sync.dma_start(x1s[:], x1_d[t, :, nn * 512:(nn + 1) * 512])
                    tmp = dorpool.tile([128, 512], F32, tag="d_tmp")
                    nc.vector.tensor_scalar_mul(tmp[:], ps[:], dqact_all[:, t:t + 1])
                    ot = dorpool.tile([128, 512], F32, tag="ot")
                    nc.gpsimd.tensor_tensor(ot[:], tmp[:], x1s[:], OP.add)
                    nc.scalar.dma_start(out_d[t * 128:(t + 1) * 128,
                                              nn * 512:(nn + 1) * 512], ot[:])
        svpool.release()
        dpool.release()
        aqA.release()
        hnA.release()
        dorpool.release()
        nwpool.release()
        perm.release()

    nc.compile()
    return nc


_NC_CACHE = {}


def _get_nc(skip_anw, skip_fnw):
    key = (skip_anw, skip_fnw)
    if key not in _NC_CACHE:
        _NC_CACHE[key] = build_program(skip_anw, skip_fnw)
    return _NC_CACHE[key]


def _ternarize(w):
    """Exact reference _weight_quant: per-tensor ternary + fp32 scale."""
    w = np.asarray(w, np.float32)
    m = np.float32(max(np.float32(np.mean(np.abs(w))), np.float32(1e-5)))
    scale = np.float32(1.0) / m
    t = np.clip(np.rint(w * scale), -1, 1)
    return t, np.float32(m)


def _host_inputs(x, attn_norm_w, ffn_norm_w, qkv_w, o_w, ffn_up_w, ffn_down_w):
    x = np.ascontiguousarray(np.asarray(x, np.float32))
    anw = np.ascontiguousarray(np.asarray(attn_norm_w, np.float32))
    fnw = np.ascontiguousarray(np.asarray(ffn_norm_w, np.float32))

    tq, m_qkv = _ternarize(np.asarray(qkv_w, np.float32).T)       # [2048, 6144]
    to, m_o = _ternarize(np.asarray(o_w, np.float32).T)           # [2048, 2048]
    td, m_dn = _ternarize(np.asarray(ffn_down_w, np.float32).T)   # [4096, 2048]

    wqkv3 = np.ascontiguousarray(
        tq.reshape(16, 128, 12, 512).transpose(2, 0, 1, 3)[QKV_ORDER]
    ).astype(ml_dtypes.bfloat16)
    wo3 = np.ascontiguousarray(
        to.reshape(16, 128, 4, 512).transpose(2, 0, 1, 3)
    ).astype(ml_dtypes.bfloat16)
    wdn3 = np.ascontiguousarray(
        td.reshape(32, 128, 4, 512).transpose(2, 0, 1, 3)
    ).astype(ml_dtypes.bfloat16)
    wup = np.asarray(ffn_up_w, np.float32).T                      # [2048, 8192]
    wup_h = np.ascontiguousarray(
        wup.reshape(16, 128, 16, 512).transpose(2, 0, 1, 3)[UP_ORDER]
    ).astype(np.float16)

    dqs = np.array([m_qkv / 127.0, m_o / 127.0, m_dn / 127.0, 0.0], np.float32)

    inv = 1.0 / (THETA ** (np.arange(0, HD, 2, dtype=np.float32) / HD))
    tpos = np.arange(S, dtype=np.float32)
    fr = np.outer(tpos, inv)                     # [S, 64]
    cosf = np.cos(fr).astype(np.float32)
    sinf = np.sin(fr).astype(np.float32)

    in_maps = []
    for c in range(NCORES):
        b = c // 4
        t0 = (c % 4) * R
        in_maps.append({
            "x_sh": np.ascontiguousarray(x[b, t0:t0 + R, :]),
            "cosb": np.ascontiguousarray(cosf[t0:t0 + R].reshape(NT, 128, 64)),
            "sinb": np.ascontiguousarray(sinf[t0:t0 + R].reshape(NT, 128, 64)),
            "attn_norm_w": anw, "ffn_norm_w": fnw, "dqs": dqs,
            "wqkv3": wqkv3, "wo3": wo3, "wup_h": wup_h, "wdn3": wdn3,
        })
    skip_anw = bool(np.all(anw == 1.0))
    skip_fnw = bool(np.all(fnw == 1.0))
    return in_maps, skip_anw, skip_fnw


def run(trace=False, **inputs):
    in_maps, skip_anw, skip_fnw = _host_inputs(**inputs)
    nc = _get_nc(skip_anw, skip_fnw)
    res = run_bass_kernel_spmd(nc, in_maps, list(range(NCORES)), trace=trace)
    out = np.empty((B, S, H), np.float32)
    for c in range(NCORES):
        b = c // 4
        t0 = (c % 4) * R
        out[b, t0:t0 + R, :] = res.results[c]["out_sh"]
    return out, res


def kernel(**inputs):
    out, _ = run(trace=False, **inputs)
    return out
